# revision 5
# baseline (speedup 1.0000x reference)
"""APPNP (gcn_norm, K=10, alpha=0.1) on 8 TRN2 NeuronCores (Bass/Tile).

Distribution: edges sharded by dst range (12500 dst rows per core); the
dinv-prescaled feature table xs = dinv * x is replicated via a per-step
AllGather (bf16, 256B rows).

Per step, per core:
  1. dma_gather (4 SWDGE queues) fetches xs[src] rows for this core's edges,
     in (dst-window, src-chunk)-grouped tiles of 128 edges.
  2. TensorE combines each tile against a host-precomputed one-hot [128, 128]
     (edge -> dst-within-window), accumulating per 128-dst window in PSUM.
  3. DVE applies the dst-side scaling: gcn norm factorizes as
     norm[e] = dinv[src]*dinv[dst]; dinv[src] is folded into the table and
     dinv[dst] into the per-row update. Self loops contribute dinv[d]*xs[d],
     handled by adding xs_slice to agg before scaling. Then the alpha mix and
     the table write-back + AllGather.

All 8 cores run one SPMD program: per-(window, chunk) tile counts are the max
across cores; cores with fewer edges pad with zero one-hot columns and idx 0.
"""
import os

import numpy as np

N = 100000
E = 1600000
D = 48
K = 10
ALPHA = 0.1
P = 128
CORES = 8
NSL = N // CORES            # 12500 dst rows per core
NW = (NSL + P - 1) // P     # 98 windows of 128 dst
NSLP = NW * P               # 12544
CHROWS = 25000              # src chunk rows (fits int16 gather indices)
NCH = N // CHROWS           # 4
SWW = 6                     # windows per super-window (PSUM has 8 banks)
MAXT = 8                    # tiles per gather call (num_idxs <= 1024)
TBL = 128                   # bf16 table row: 48 feats + pad -> 256B

LAST_EXEC_NS = None


def _wrap_idx(idx16):
    """[ni] -> [128, ni//16] int16: slot i at partition i%16, col i//16, x8 groups."""
    ni = idx16.shape[0]
    blk = idx16.reshape(ni // 16, 16).T.astype(np.int16)
    return np.tile(blk, (8, 1))


def _prep(src, dst):
    """Group each core's edges by (dst window, src chunk); canonical tile
    counts = max over cores. Returns schedule + per-core gidx/oh arrays."""
    per_core = []  # core -> dict[(w, ch)] -> (src_local array, dstrel array)
    for c in range(CORES):
        lo = c * NSL
        sel = (dst >= lo) & (dst < lo + NSL)
        es = src[sel]
        ed = dst[sel] - lo
        w = ed >> 7
        ch = es // CHROWS
        order = np.lexsort((es, ch, w))
        es, ed, w, ch = es[order], ed[order], w[order], ch[order]
        key = w * NCH + ch
        bounds = np.flatnonzero(np.diff(key)) + 1
        starts = np.concatenate(([0], bounds))
        ends = np.concatenate((bounds, [len(es)]))
        groups = {}
        for s, e in zip(starts, ends):
            groups[(int(w[s]), int(ch[s]))] = (es[s:e] - int(ch[s]) * CHROWS,
                                               ed[s:e] - int(w[s]) * P)
        per_core.append(groups)

    nt_canon = {}
    for wdw in range(NW):
        for ch in range(NCH):
            m = 0
            for c in range(CORES):
                g = per_core[c].get((wdw, ch))
                if g is not None:
                    m = max(m, (len(g[0]) + P - 1) // P)
            nt_canon[(wdw, ch)] = m

    # canonical execution order: sw asc -> chunk asc -> window asc -> tile asc,
    # grouped into gather calls of <= MAXT tiles (single chunk per call).
    tile_window = []
    call_meta = []   # (chunk, ntiles, tile_pos0)
    tile_src_of = []  # (w, ch, k) per canonical tile position
    for sw0 in range(0, NW, SWW):
        for ch in range(NCH):
            run = []
            for wdw in range(sw0, min(sw0 + SWW, NW)):
                for k in range(nt_canon[(wdw, ch)]):
                    run.append((wdw, ch, k))
            for s in range(0, len(run), MAXT):
                chunk_run = run[s:s + MAXT]
                call_meta.append((ch, len(chunk_run), len(tile_window)))
                for (wdw, ch_, k) in chunk_run:
                    tile_src_of.append((wdw, ch_, k))
                    tile_window.append(wdw)

    ntiles = len(tile_window)
    first_of, last_of = {}, {}
    for pos, wdw in enumerate(tile_window):
        if wdw not in first_of:
            first_of[wdw] = pos
        last_of[wdw] = pos
    tile_start = [pos == first_of[w_] for pos, w_ in enumerate(tile_window)]
    tile_stop = [pos == last_of[w_] for pos, w_ in enumerate(tile_window)]

    # per-core arrays in canonical order
    gidx = np.zeros((CORES, P, ntiles * P // 16), np.int16)
    oh = np.zeros((CORES, ntiles, P, P), np.float32)
    for c in range(CORES):
        flat_idx = np.zeros((ntiles, P), np.int16)
        for pos, (wdw, ch, k) in enumerate(tile_src_of):
            g = per_core[c].get((wdw, ch))
            if g is None:
                continue
            gs, gr = g
            seg_s = k * P
            seg = gs[seg_s:seg_s + P]
            rel = gr[seg_s:seg_s + P]
            if len(seg) == 0:
                continue
            flat_idx[pos, :len(seg)] = seg.astype(np.int16)
            oh[c, pos, np.arange(len(rel)), rel] = 1.0
        # wrap per call (contiguous tile runs share one idx region)
        col = 0
        for (_ch, nt, pos0) in call_meta:
            ni = nt * P
            gidx[c, :, col:col + ni // 16] = _wrap_idx(flat_idx[pos0:pos0 + nt].reshape(-1))
            col += ni // 16
    return {
        "call_meta": call_meta,
        "tile_window": tile_window,
        "tile_start": tile_start,
        "tile_stop": tile_stop,
        "ntiles": ntiles,
        "gidx": gidx,
        "oh": oh,
    }


def kernel(x, x_0, edge_index):
    global LAST_EXEC_NS
    import ml_dtypes
    import concourse.bacc as bacc
    import concourse.mybir as mybir
    import concourse.tile as tile
    from concourse.bass_utils import run_bass_kernel_spmd

    bf16 = ml_dtypes.bfloat16
    x = np.asarray(x, np.float32)
    src = np.asarray(edge_index[0], np.int64)
    dst = np.asarray(edge_index[1], np.int64)

    deg = np.bincount(dst, minlength=N).astype(np.float32) + 1.0
    dinv = (1.0 / np.sqrt(deg)).astype(np.float32)

    xs0 = np.zeros((N, TBL), np.float32)
    xs0[:, :D] = x * dinv[:, None]
    xs0_bf = xs0.astype(bf16)

    sched = _prep(src, dst)
    NT = sched["ntiles"]
    call_meta = sched["call_meta"]
    tile_window = sched["tile_window"]
    tile_start = sched["tile_start"]
    tile_stop = sched["tile_stop"]
    oh_np = sched["oh"].astype(bf16)
    gidx_np = sched["gidx"]

    xsl0 = np.zeros((CORES, P, NW, D), np.float32)
    h01 = np.zeros((CORES, P, NW, D), np.float32)
    dinv_t = np.zeros((CORES, P, NW, 1), np.float32)
    for c in range(CORES):
        rows = np.arange(c * NSL, (c + 1) * NSL)
        pw = np.arange(NSL)
        pp, ww = pw % P, pw // P
        xsl0[c, pp, ww] = xs0[rows, :D]
        h01[c, pp, ww] = ALPHA * x[rows]
        dinv_t[c, pp, ww, 0] = dinv[rows]

    nc = bacc.Bacc("TRN2", target_bir_lowering=False, debug=False,
                   num_devices=CORES, num_swdge_queues=4)
    dt = mybir.dt

    t_xs0 = nc.dram_tensor("xs0", [N, TBL], dt.bfloat16, kind="ExternalInput")
    t_gidx = nc.dram_tensor("gidx", [P, gidx_np.shape[2]], dt.int16, kind="ExternalInput")
    t_oh = nc.dram_tensor("oh", [NT, P, P], dt.bfloat16, kind="ExternalInput")
    t_xsl = nc.dram_tensor("xsl", [P, NW, D], dt.float32, kind="ExternalInput")
    t_h01 = nc.dram_tensor("h01", [P, NW, D], dt.float32, kind="ExternalInput")
    t_dinv = nc.dram_tensor("dinvt", [P, NW, 1], dt.float32, kind="ExternalInput")
    t_out = nc.dram_tensor("out", [NSLP, D], dt.float32, kind="ExternalOutput")
    t_bounce = nc.dram_tensor("bounce", [NSL, TBL], dt.bfloat16)
    t_tbl = [nc.dram_tensor(f"tbl{i}", [N, TBL], dt.bfloat16, addr_space="Shared")
             for i in range(2)]

    with tile.TileContext(nc) as tc:
        with tc.tile_pool(name="const", bufs=1) as constp, \
             tc.tile_pool(name="gb", bufs=6) as gbp, \
             tc.tile_pool(name="ohp", bufs=6) as ohp, \
             tc.tile_pool(name="psum", bufs=8, space="PSUM") as psp, \
             tc.tile_pool(name="upd", bufs=1) as updp:

            gidx_sb = constp.tile([P, gidx_np.shape[2]], dt.int16)
            nc.sync.dma_start(out=gidx_sb[:], in_=t_gidx[:])

            agg = updp.tile([P, NW, D], dt.float32)
            nc.vector.memset(agg[:], 0.0)
            xsl = updp.tile([P, NW, D], dt.float32)
            nc.sync.dma_start(out=xsl[:], in_=t_xsl[:])
            h01sb = updp.tile([P, NW, D], dt.float32)
            nc.sync.dma_start(out=h01sb[:], in_=t_h01[:])
            dinvsb = updp.tile([P, NW, 1], dt.float32)
            nc.sync.dma_start(out=dinvsb[:], in_=t_dinv[:])
            dinv09 = updp.tile([P, NW, 1], dt.float32)
            nc.scalar.mul(out=dinv09[:], in_=dinvsb[:], mul=1.0 - ALPHA)
            xnext = updp.tile([P, NW, D], dt.float32)
            xsb = updp.tile([P, NW, TBL], dt.bfloat16)
            nc.vector.memset(xsb[:], 0.0)

            for step in range(K):
                src_tbl = t_xs0 if step == 0 else t_tbl[(step + 1) % 2]
                psum_of_w = {}
                for ci, (gch, ntile, pos0) in enumerate(call_meta):
                    ni = ntile * P
                    coloff = pos0 * P // 16
                    gb = gbp.tile([P, MAXT, TBL], dt.bfloat16, tag="gb")
                    nc.gpsimd.dma_gather(
                        gb[:, :ntile, :],
                        src_tbl[gch * CHROWS:(gch + 1) * CHROWS, :],
                        gidx_sb[:, coloff:coloff + ni // 16],
                        ni, ni, TBL,
                        queue_num=ci % 4,
                    )
                    ohb = ohp.tile([P, MAXT, P], dt.bfloat16, tag="ohb")
                    nc.sync.dma_start(
                        out=ohb[:, :ntile, :],
                        in_=t_oh[pos0:pos0 + ntile].rearrange("t p d -> p t d"),
                    )
                    for k in range(ntile):
                        pos = pos0 + k
                        wdw = tile_window[pos]
                        if wdw not in psum_of_w:
                            psum_of_w[wdw] = psp.tile([P, D], dt.float32, space="PSUM", tag="pw", name=f"ps_{step}_{wdw}")
                        nc.tensor.matmul(
                            out=psum_of_w[wdw][:],
                            lhsT=ohb[:, k, :],
                            rhs=gb[:, k, :D],
                            start=tile_start[pos],
                            stop=tile_stop[pos],
                        )
                        if tile_stop[pos]:
                            nc.vector.tensor_copy(out=agg[:, wdw, :], in_=psum_of_w[wdw][:])
                            del psum_of_w[wdw]

                nc.vector.tensor_add(out=agg[:], in0=agg[:], in1=xsl[:])
                nc.vector.tensor_mul(out=xnext[:], in0=agg[:],
                                     in1=dinv09[:].to_broadcast([P, NW, D]))
                nc.vector.tensor_add(out=xnext[:], in0=xnext[:], in1=h01sb[:])
                if step < K - 1:
                    nc.vector.tensor_mul(out=xsl[:], in0=xnext[:],
                                         in1=dinvsb[:].to_broadcast([P, NW, D]))
                    nc.scalar.copy(out=xsb[:, :, :D], in_=xsl[:])
                    fw = NSL // P        # 97 full windows
                    rem = NSL - fw * P   # 84
                    nc.sync.dma_start(
                        out=t_bounce[:fw * P, :].rearrange("(w p) f -> p w f", p=P),
                        in_=xsb[:, :fw, :],
                    )
                    if rem:
                        nc.sync.dma_start(
                            out=t_bounce[fw * P:, :].rearrange("(w p) f -> p w f", p=rem),
                            in_=xsb[:rem, fw:fw + 1, :],
                        )
                    nc.gpsimd.collective_compute(
                        "AllGather",
                        mybir.AluOpType.bypass,
                        replica_groups=[list(range(CORES))],
                        ins=[t_bounce[:].opt()],
                        outs=[t_tbl[step % 2][:].opt()],
                    )

            nc.sync.dma_start(out=t_out[:].rearrange("(w p) f -> p w f", p=P),
                              in_=xnext[:])
    nc.compile()

    in_maps = []
    for c in range(CORES):
        in_maps.append({
            "xs0": xs0_bf,
            "gidx": gidx_np[c],
            "oh": oh_np[c],
            "xsl": xsl0[c],
            "h01": h01[c],
            "dinvt": dinv_t[c],
        })
    do_trace = os.environ.get("APPNP_TRACE", "0") == "1"
    res = run_bass_kernel_spmd(nc, in_maps, core_ids=list(range(CORES)), trace=do_trace)
    LAST_EXEC_NS = res.exec_time_ns
    if res.exec_time_ns:
        print(f"HW exec time: {res.exec_time_ns} ns")
    out = np.zeros((N, D), np.float32)
    for c in range(CORES):
        out[c * NSL:(c + 1) * NSL] = res.results[c]["out"][:NSL]
    return out


# revision 6
# speedup vs baseline: 1.0563x; 1.0563x over previous
"""APPNP (gcn_norm, K=10, alpha=0.1) on 8 TRN2 NeuronCores (Bass/Tile).

Distribution: edges sharded by dst range (12500 dst rows per core); the
dinv-prescaled feature table xs = dinv * x is replicated via a per-step
AllGather (bf16, 256B rows).

Per step, per core:
  1. dma_gather (4 SWDGE queues) fetches xs[src] rows for this core's edges,
     in (dst-window, src-chunk)-grouped tiles of 128 edges.
  2. TensorE combines each tile against a host-precomputed one-hot [128, 128]
     (edge -> dst-within-window), accumulating per 128-dst window in PSUM.
  3. DVE applies the dst-side scaling: gcn norm factorizes as
     norm[e] = dinv[src]*dinv[dst]; dinv[src] is folded into the table and
     dinv[dst] into the per-row update. Self loops contribute dinv[d]*xs[d],
     handled by adding xs_slice to agg before scaling. Then the alpha mix and
     the table write-back + AllGather.

All 8 cores run one SPMD program: per-(window, chunk) tile counts are the max
across cores; cores with fewer edges pad with zero one-hot columns and idx 0.
"""
import os

import numpy as np

N = 100000
E = 1600000
D = 48
K = 10
ALPHA = 0.1
P = 128
CORES = 8
NSL = N // CORES            # 12500 dst rows per core
NW = (NSL + P - 1) // P     # 98 windows of 128 dst
NSLP = NW * P               # 12544
NTBL = CORES * NSLP         # padded table rows (p-major per core)
CHROWS = NTBL // 4          # src chunk rows (fits int16 gather indices)
NCH = 4
SWW = 6                     # windows per super-window (PSUM has 8 banks)
MAXT = 16                   # tiles per gather call (num_idxs <= 2048)
TBL = 128                   # bf16 table row: 48 feats + pad -> 256B


def _tbl_row(n):
    """table row of node n: p-major within each core's slice so the table
    write-back from SBUF [p, w, f] is one contiguous-per-partition DMA."""
    c = n // NSL
    j = n - c * NSL
    return c * NSLP + (j % P) * NW + j // P

LAST_EXEC_NS = None


def _wrap_idx(idx16):
    """[ni] -> [128, ni//16] int16: slot i at partition i%16, col i//16, x8 groups."""
    ni = idx16.shape[0]
    blk = idx16.reshape(ni // 16, 16).T.astype(np.int16)
    return np.tile(blk, (8, 1))


def _prep(src, dst):
    """Group each core's edges by (dst window, src chunk); canonical tile
    counts = max over cores. Returns schedule + per-core gidx/oh arrays."""
    per_core = []  # core -> dict[(w, ch)] -> (src_local array, dstrel array)
    for c in range(CORES):
        lo = c * NSL
        sel = (dst >= lo) & (dst < lo + NSL)
        es = _tbl_row(src[sel])
        ed = dst[sel] - lo
        w = ed >> 7
        ch = es // CHROWS
        order = np.lexsort((es, ch, w))
        es, ed, w, ch = es[order], ed[order], w[order], ch[order]
        key = w * NCH + ch
        bounds = np.flatnonzero(np.diff(key)) + 1
        starts = np.concatenate(([0], bounds))
        ends = np.concatenate((bounds, [len(es)]))
        groups = {}
        for s, e in zip(starts, ends):
            groups[(int(w[s]), int(ch[s]))] = (es[s:e] - int(ch[s]) * CHROWS,
                                               ed[s:e] - int(w[s]) * P)
        per_core.append(groups)

    nt_canon = {}
    for wdw in range(NW):
        for ch in range(NCH):
            m = 0
            for c in range(CORES):
                g = per_core[c].get((wdw, ch))
                if g is not None:
                    m = max(m, (len(g[0]) + P - 1) // P)
            nt_canon[(wdw, ch)] = m

    # canonical execution order: sw asc -> chunk asc -> window asc -> tile asc,
    # grouped into gather calls of <= MAXT tiles (single chunk per call).
    tile_window = []
    call_meta = []   # (chunk, ntiles, tile_pos0)
    tile_src_of = []  # (w, ch, k) per canonical tile position
    for sw0 in range(0, NW, SWW):
        for ch in range(NCH):
            run = []
            for wdw in range(sw0, min(sw0 + SWW, NW)):
                for k in range(nt_canon[(wdw, ch)]):
                    run.append((wdw, ch, k))
            for s in range(0, len(run), MAXT):
                chunk_run = run[s:s + MAXT]
                call_meta.append((ch, len(chunk_run), len(tile_window)))
                for (wdw, ch_, k) in chunk_run:
                    tile_src_of.append((wdw, ch_, k))
                    tile_window.append(wdw)

    ntiles = len(tile_window)
    first_of, last_of = {}, {}
    for pos, wdw in enumerate(tile_window):
        if wdw not in first_of:
            first_of[wdw] = pos
        last_of[wdw] = pos
    tile_start = [pos == first_of[w_] for pos, w_ in enumerate(tile_window)]
    tile_stop = [pos == last_of[w_] for pos, w_ in enumerate(tile_window)]

    # per-core arrays in canonical order
    gidx = np.zeros((CORES, P, ntiles * P // 16), np.int16)
    oh = np.zeros((CORES, ntiles, P, P), np.float32)
    for c in range(CORES):
        flat_idx = np.zeros((ntiles, P), np.int16)
        for pos, (wdw, ch, k) in enumerate(tile_src_of):
            g = per_core[c].get((wdw, ch))
            if g is None:
                continue
            gs, gr = g
            seg_s = k * P
            seg = gs[seg_s:seg_s + P]
            rel = gr[seg_s:seg_s + P]
            if len(seg) == 0:
                continue
            flat_idx[pos, :len(seg)] = seg.astype(np.int16)
            oh[c, pos, np.arange(len(rel)), rel] = 1.0
        # wrap per call (contiguous tile runs share one idx region)
        col = 0
        for (_ch, nt, pos0) in call_meta:
            ni = nt * P
            gidx[c, :, col:col + ni // 16] = _wrap_idx(flat_idx[pos0:pos0 + nt].reshape(-1))
            col += ni // 16
    return {
        "call_meta": call_meta,
        "tile_window": tile_window,
        "tile_start": tile_start,
        "tile_stop": tile_stop,
        "ntiles": ntiles,
        "gidx": gidx,
        "oh": oh,
    }


def kernel(x, x_0, edge_index):
    global LAST_EXEC_NS
    import ml_dtypes
    import concourse.bacc as bacc
    import concourse.mybir as mybir
    import concourse.tile as tile
    from concourse.bass_utils import run_bass_kernel_spmd

    bf16 = ml_dtypes.bfloat16
    x = np.asarray(x, np.float32)
    src = np.asarray(edge_index[0], np.int64)
    dst = np.asarray(edge_index[1], np.int64)

    deg = np.bincount(dst, minlength=N).astype(np.float32) + 1.0
    dinv = (1.0 / np.sqrt(deg)).astype(np.float32)

    xs0 = np.zeros((NTBL, TBL), np.float32)
    xs0[_tbl_row(np.arange(N)), :D] = x * dinv[:, None]
    xs0_bf = xs0.astype(bf16)

    sched = _prep(src, dst)
    NT = sched["ntiles"]
    call_meta = sched["call_meta"]
    tile_window = sched["tile_window"]
    tile_start = sched["tile_start"]
    tile_stop = sched["tile_stop"]
    oh_np = np.ascontiguousarray(sched["oh"].transpose(0, 2, 1, 3)).astype(bf16)  # [C, P, NT, P]
    gidx_np = sched["gidx"]

    xsl0 = np.zeros((CORES, P, NW, D), np.float32)
    h01 = np.zeros((CORES, P, NW, D), np.float32)
    dinv_t = np.zeros((CORES, P, NW, 1), np.float32)
    for c in range(CORES):
        rows = np.arange(c * NSL, (c + 1) * NSL)
        pw = np.arange(NSL)
        pp, ww = pw % P, pw // P
        xsl0[c, pp, ww] = xs0[_tbl_row(rows), :D]
        h01[c, pp, ww] = ALPHA * x[rows]
        dinv_t[c, pp, ww, 0] = dinv[rows]

    nc = bacc.Bacc("TRN2", target_bir_lowering=False, debug=False,
                   num_devices=CORES, num_swdge_queues=4)
    dt = mybir.dt

    t_xs0 = nc.dram_tensor("xs0", [NTBL, TBL], dt.bfloat16, kind="ExternalInput")
    t_gidx = nc.dram_tensor("gidx", [P, gidx_np.shape[2]], dt.int16, kind="ExternalInput")
    t_oh = nc.dram_tensor("oh", [P, NT, P], dt.bfloat16, kind="ExternalInput")
    t_xsl = nc.dram_tensor("xsl", [P, NW, D], dt.float32, kind="ExternalInput")
    t_h01 = nc.dram_tensor("h01", [P, NW, D], dt.float32, kind="ExternalInput")
    t_dinv = nc.dram_tensor("dinvt", [P, NW, 1], dt.float32, kind="ExternalInput")
    t_out = nc.dram_tensor("out", [NSLP, D], dt.float32, kind="ExternalOutput")
    t_bounce = nc.dram_tensor("bounce", [NSLP, TBL], dt.bfloat16)
    t_tbl = [nc.dram_tensor(f"tbl{i}", [NTBL, TBL], dt.bfloat16, addr_space="Shared")
             for i in range(2)]

    with tile.TileContext(nc) as tc:
        with tc.tile_pool(name="const", bufs=1) as constp, \
             tc.tile_pool(name="gb", bufs=6) as gbp, \
             tc.tile_pool(name="ohp", bufs=6) as ohp, \
             tc.tile_pool(name="psum", bufs=8, space="PSUM") as psp, \
             tc.tile_pool(name="upd", bufs=1) as updp:

            gidx_sb = constp.tile([P, gidx_np.shape[2]], dt.int16)
            nc.sync.dma_start(out=gidx_sb[:], in_=t_gidx[:])

            agg = updp.tile([P, NW, D], dt.float32)
            nc.vector.memset(agg[:], 0.0)
            xsl = updp.tile([P, NW, D], dt.float32)
            nc.sync.dma_start(out=xsl[:], in_=t_xsl[:])
            h01sb = updp.tile([P, NW, D], dt.float32)
            nc.sync.dma_start(out=h01sb[:], in_=t_h01[:])
            dinvsb = updp.tile([P, NW, 1], dt.float32)
            nc.sync.dma_start(out=dinvsb[:], in_=t_dinv[:])
            dinv09 = updp.tile([P, NW, 1], dt.float32)
            nc.scalar.mul(out=dinv09[:], in_=dinvsb[:], mul=1.0 - ALPHA)
            xnext = updp.tile([P, NW, D], dt.float32)
            xsb = updp.tile([P, NW, TBL], dt.bfloat16)
            nc.vector.memset(xsb[:], 0.0)

            for step in range(K):
                src_tbl = t_xs0 if step == 0 else t_tbl[(step + 1) % 2]
                psum_of_w = {}
                for ci, (gch, ntile, pos0) in enumerate(call_meta):
                    ni = ntile * P
                    coloff = pos0 * P // 16
                    gb = gbp.tile([P, MAXT, TBL], dt.bfloat16, tag="gb")
                    nc.gpsimd.dma_gather(
                        gb[:, :ntile, :],
                        src_tbl[gch * CHROWS:(gch + 1) * CHROWS, :],
                        gidx_sb[:, coloff:coloff + ni // 16],
                        ni, ni, TBL,
                        queue_num=ci % 4,
                        single_packet=False,
                    )
                    ohb = ohp.tile([P, MAXT, P], dt.bfloat16, tag="ohb")
                    nc.scalar.dma_start(
                        out=ohb[:, :ntile, :],
                        in_=t_oh[:, pos0:pos0 + ntile, :],
                    )
                    for k in range(ntile):
                        pos = pos0 + k
                        wdw = tile_window[pos]
                        if wdw not in psum_of_w:
                            psum_of_w[wdw] = psp.tile([P, D], dt.float32, space="PSUM", tag="pw", name=f"ps_{step}_{wdw}")
                        nc.tensor.matmul(
                            out=psum_of_w[wdw][:],
                            lhsT=ohb[:, k, :],
                            rhs=gb[:, k, :D],
                            start=tile_start[pos],
                            stop=tile_stop[pos],
                        )
                        if tile_stop[pos]:
                            nc.vector.tensor_copy(out=agg[:, wdw, :], in_=psum_of_w[wdw][:])
                            del psum_of_w[wdw]

                nc.vector.tensor_add(out=agg[:], in0=agg[:], in1=xsl[:])
                nc.vector.tensor_mul(out=xnext[:], in0=agg[:],
                                     in1=dinv09[:].to_broadcast([P, NW, D]))
                nc.vector.tensor_add(out=xnext[:], in0=xnext[:], in1=h01sb[:])
                if step < K - 1:
                    nc.vector.tensor_mul(out=xsl[:], in0=xnext[:],
                                         in1=dinvsb[:].to_broadcast([P, NW, D]))
                    nc.scalar.copy(out=xsb[:, :, :D], in_=xsl[:])
                    nc.sync.dma_start(
                        out=t_bounce[:].rearrange("(p w) f -> p w f", p=P),
                        in_=xsb[:],
                    )
                    nc.gpsimd.collective_compute(
                        "AllGather",
                        mybir.AluOpType.bypass,
                        replica_groups=[list(range(CORES))],
                        ins=[t_bounce[:].opt()],
                        outs=[t_tbl[step % 2][:].opt()],
                    )

            nc.sync.dma_start(out=t_out[:].rearrange("(w p) f -> p w f", p=P),
                              in_=xnext[:])
    nc.compile()

    in_maps = []
    for c in range(CORES):
        in_maps.append({
            "xs0": xs0_bf,
            "gidx": gidx_np[c],
            "oh": oh_np[c],
            "xsl": xsl0[c],
            "h01": h01[c],
            "dinvt": dinv_t[c],
        })
    do_trace = os.environ.get("APPNP_TRACE", "0") == "1"
    res = run_bass_kernel_spmd(nc, in_maps, core_ids=list(range(CORES)), trace=do_trace)
    LAST_EXEC_NS = res.exec_time_ns
    if res.exec_time_ns:
        print(f"HW exec time: {res.exec_time_ns} ns")
    out = np.zeros((N, D), np.float32)
    for c in range(CORES):
        out[c * NSL:(c + 1) * NSL] = res.results[c]["out"][:NSL]
    return out
